# revision 1
# baseline (speedup 1.0000x reference)
"""MLA encoder self-attention on 8 TRN2 NeuronCores.

Sharding: data-parallel over batch (B=2) x tensor-parallel over head groups
(16 heads -> 4 groups of 4). Core c handles batch c//4, heads 4*(c%4)..+4.
Each core computes its heads' attention and a partial output projection;
the host sums the 4 head-group partials per batch.

All matmuls run in float32r (TF32-like, full-rate on the PE); accumulation
is fp32 in PSUM. End-to-end rel err vs the f32 reference ~6e-4.

Layout notes:
- Activations flow token-major (tokens on partitions) for LayerNorm/rope,
  then are PE-transposed to feature-major for the next contraction.
- kvd layout per head: [nope(32) | v(64) | ones(1)] stride 97, so v_aug
  ([v | 1]) is a contiguous 65-column slice: the AV matmul computes both
  the attention numerator and the softmax denominator (ones row) at once.
- Softmax has no max-subtraction (scores are bounded ~2, verified on host
  data) and normalization is deferred: out_aug^T rows 0..63 are scaled by
  1/Z (row 64) right before the output projection.
"""
import numpy as np

B, S, E = 2, 2048, 1024
H, HD = 16, 64
ROPE, NOPE = 32, 32
QL, KVL = 512, 512
EPS = 1e-5
HPC = 4                # heads per core
DPC = HPC * HD         # 256
NCORES = 8
TT = S // 128          # 16 token tiles
ET = E // 128          # 8
RT = QL // 128         # 4
SB = S // 512          # 4 s-blocks
UT = S // 128          # 16 key tiles
KVW = 4 * 97           # 388: kvd width

_CACHE = {}


def _build(reps=1):
    import concourse.tile as tile
    import concourse.mybir as mybir
    from concourse import bacc

    f32 = mybir.dt.float32
    f32r = mybir.dt.float32r
    AF = mybir.ActivationFunctionType
    ALU = mybir.AluOpType

    nc = bacc.Bacc("TRN2", target_bir_lowering=False, debug=False,
                   num_devices=NCORES)

    def din(name, shape, dt=f32r):
        return nc.dram_tensor(name, shape, dt, kind="ExternalInput").ap()

    xT_d = din("xT", (E, S))
    wqa_d = din("WqaT", (E, QL))
    wkva_d = din("WkvaT", (E, KVL + ROPE))
    wqb_d = din("WqbT", (QL, DPC))
    qconst_d = din("qconst", (1, DPC))
    wkvb_d = din("WkvbT", (KVL, KVW))
    kconst_d = din("kconst", (1, KVW))
    wout_d = din("WoutT", (DPC, E))
    aq_d = din("Aq", (S, DPC))
    bq_d = din("Bq", (S, DPC))
    c2k_d = din("c2k", (S, ROPE))
    s2k_d = din("s2k", (S, ROPE))
    ident_d = din("ident", (128, 128))
    ones1_d = din("ones1", (1, 128))
    ones4_d = din("ones4", (128, 4))
    eps_d = din("epst", (128, 1), mybir.dt.float32)
    out_d = nc.dram_tensor("out", (S, E), f32, kind="ExternalOutput").ap()

    with tile.TileContext(nc) as tc:
        import contextlib
        with contextlib.ExitStack() as top:
            consts = top.enter_context(tc.tile_pool(name="consts", bufs=1))
            ident_t = consts.tile([128, 128], f32r, tag="ident")
            nc.sync.dma_start(ident_t[:], ident_d[:])
            ones1_t = consts.tile([1, 128], f32r, tag="ones1")
            nc.sync.dma_start(ones1_t[:], ones1_d[:])
            ones4_t = consts.tile([128, 4], f32r, tag="ones4")
            nc.sync.dma_start(ones4_t[:], ones4_d[:])
            qconst_t = consts.tile([1, DPC], f32r, tag="qconst")
            nc.sync.dma_start(qconst_t[:], qconst_d[:])
            kconst_t = consts.tile([1, KVW], f32r, tag="kconst")
            nc.sync.dma_start(kconst_t[:], kconst_d[:])
            eps_t = consts.tile([128, 1], f32, tag="epst")
            nc.sync.dma_start(eps_t[:], eps_d[:])

            acts = top.enter_context(tc.tile_pool(name="acts", bufs=1))
            wq2 = top.enter_context(tc.tile_pool(name="wq2", bufs=1))

            def body():
                qcnT = [acts.tile([128, S], f32r, tag=f"qcnT{r}", name=f"qcnT{r}")
                        for r in range(RT)]
                ckvnT = [acts.tile([128, S], f32r, tag=f"ckvnT{r}", name=f"ckvnT{r}")
                        for r in range(RT)]
                kpe = [acts.tile([128, ROPE], f32r, tag=f"kpe{t}", name=f"kpe{t}")
                       for t in range(TT)]

                # ---------------- stage 1: qa + kva + LN + kpe rope -------
                with contextlib.ExitStack() as st1:
                    wqa_p = st1.enter_context(tc.tile_pool(name="wqa", bufs=1))
                    wqa_t = wqa_p.tile([128, ET, QL], f32r, tag="wqa")
                    wkva_t = wqa_p.tile([128, ET, KVL + ROPE], f32r, tag="wkva")

                    pmm = st1.enter_context(
                        tc.tile_pool(name="pmm", bufs=2, space="PSUM"))
                    ptp = st1.enter_context(
                        tc.tile_pool(name="ptp", bufs=2, space="PSUM"))
                    xp = st1.enter_context(tc.tile_pool(name="xp", bufs=3))
                    sp = st1.enter_context(tc.tile_pool(name="sp", bufs=3))
                    ck = st1.enter_context(tc.tile_pool(name="ck", bufs=1))
                    c2k_t = ck.tile([128, TT, ROPE], f32r, tag="c2k")
                    nc.sync.dma_start(
                        c2k_t[:], c2k_d.rearrange("(t p) r -> p t r", p=128))
                    s2k_t = ck.tile([128, TT, ROPE], f32r, tag="s2k")
                    nc.sync.dma_start(
                        s2k_t[:], s2k_d.rearrange("(t p) r -> p t r", p=128))

                    pend = []

                    def do_tp1(t, qcn_t, ckvn_t):
                        ts_ = slice(t * 128, (t + 1) * 128)
                        for r in range(RT):
                            rs = slice(r * 128, (r + 1) * 128)
                            tp1 = ptp.tile([128, 128], f32r, tag="tp",
                                           name=f"tp1_{t}_{r}")
                            nc.tensor.transpose(tp1[:], qcn_t[:, rs], ident_t[:])
                            nc.scalar.activation(qcnT[r][:, ts_], tp1[:], AF.Copy)
                            tp2 = ptp.tile([128, 128], f32r, tag="tp",
                                           name=f"tp2_{t}_{r}")
                            nc.tensor.transpose(tp2[:], ckvn_t[:, rs], ident_t[:])
                            nc.vector.tensor_copy(ckvnT[r][:, ts_], tp2[:])

                    for t in range(TT):
                        ts_ = slice(t * 128, (t + 1) * 128)
                        p_qc = pmm.tile([128, QL], f32, tag="p_qc")
                        p_ka = pmm.tile([128, 272], f32, tag="p_ka")
                        p_kb = pmm.tile([128, 272], f32, tag="p_kb")
                        xt = xp.tile([128, ET, 128], f32r, tag="xt")
                        if t == 0:
                            for e in range(ET):
                                es_ = slice(e * 128, (e + 1) * 128)
                                nc.sync.dma_start(
                                    xt[:, e, :], xT_d[es_, ts_])
                                nc.sync.dma_start(wqa_t[:, e, :], wqa_d[es_, :])
                                nc.sync.dma_start(wkva_t[:, e, :],
                                                  wkva_d[es_, :])
                        else:
                            nc.sync.dma_start(
                                xt[:],
                                xT_d.rearrange("(k p) s -> p k s", p=128)[:, :, ts_])
                        for e in range(ET):
                            st, sp_ = (e == 0), (e == ET - 1)
                            nc.tensor.matmul(p_qc[:], xt[:, e, :], wqa_t[:, e, :],
                                             start=st, stop=sp_)
                            nc.tensor.matmul(p_ka[:], xt[:, e, :],
                                             wkva_t[:, e, 0:272],
                                             start=st, stop=sp_)
                            nc.tensor.matmul(p_kb[:], xt[:, e, :],
                                             wkva_t[:, e, 272:544],
                                             start=st, stop=sp_)
                        # LN on qc
                        stq = sp.tile([128, 6], f32, tag="stq")
                        nc.vector.bn_stats(stq[:], p_qc[:])
                        mvq = sp.tile([128, 2], f32, tag="mvq")
                        nc.vector.bn_aggr(mvq[:], stq[:])
                        rsq = sp.tile([128, 1], f32, tag="rsq")
                        nc.scalar.activation(rsq[:], mvq[:, 1:2], AF.Sqrt,
                                             bias=eps_t[:])
                        rsq2 = sp.tile([128, 1], f32, tag="rsq2")
                        nc.vector.reciprocal(rsq2[:], rsq[:])
                        nmq = sp.tile([128, 1], f32, tag="nmq")
                        nc.vector.tensor_scalar(
                            out=nmq[:], in0=mvq[:, 0:1], scalar1=rsq2[:],
                            scalar2=-1.0, op0=ALU.mult, op1=ALU.mult)
                        qcn_t = sp.tile([128, QL], f32r, tag="qcn", bufs=4)
                        nc.scalar.activation(qcn_t[:], p_qc[:], AF.Identity,
                                             bias=nmq[:], scale=rsq2[:])
                        # LN on ckv (272 + 240 chunks)
                        stk = sp.tile([128, 2, 6], f32, tag="stk")
                        nc.vector.bn_stats(stk[:, 0, :], p_ka[:])
                        nc.vector.bn_stats(stk[:, 1, :], p_kb[:, 0:240])
                        mvk = sp.tile([128, 2], f32, tag="mvk")
                        nc.vector.bn_aggr(mvk[:], stk[:])
                        rsk = sp.tile([128, 1], f32, tag="rsk")
                        nc.scalar.activation(rsk[:], mvk[:, 1:2], AF.Sqrt,
                                             bias=eps_t[:])
                        rsk2 = sp.tile([128, 1], f32, tag="rsk2")
                        nc.vector.reciprocal(rsk2[:], rsk[:])
                        nmk = sp.tile([128, 1], f32, tag="nmk")
                        nc.vector.tensor_scalar(
                            out=nmk[:], in0=mvk[:, 0:1], scalar1=rsk2[:],
                            scalar2=-1.0, op0=ALU.mult, op1=ALU.mult)
                        ckvn_t = sp.tile([128, KVL], f32r, tag="ckvn", bufs=4)
                        nc.scalar.activation(ckvn_t[:, 0:272], p_ka[:],
                                             AF.Identity, bias=nmk[:],
                                             scale=rsk2[:])
                        nc.scalar.activation(ckvn_t[:, 272:512], p_kb[:, 0:240],
                                             AF.Identity, bias=nmk[:],
                                             scale=rsk2[:])
                        # kpe rope (raw cols 240:272 of p_kb)
                        kraw = sp.tile([128, ROPE], f32r, tag="kraw")
                        nc.vector.tensor_copy(kraw[:], p_kb[:, 240:272])
                        ksw = sp.tile([128, ROPE], f32r, tag="ksw")
                        kraw3 = kraw.rearrange("p (i two) -> p i two", two=2)
                        ksw3 = ksw.rearrange("p (i two) -> p i two", two=2)
                        nc.gpsimd.tensor_copy(ksw3[:, :, 0:1], kraw3[:, :, 1:2])
                        nc.gpsimd.tensor_copy(ksw3[:, :, 1:2], kraw3[:, :, 0:1])
                        c2t = c2k_t[:, t, :]
                        s2t = s2k_t[:, t, :]
                        kp1 = sp.tile([128, ROPE], f32r, tag="kp1")
                        nc.gpsimd.tensor_mul(kp1[:], kraw[:], c2t[:])
                        kp2 = sp.tile([128, ROPE], f32r, tag="kp2")
                        nc.gpsimd.tensor_mul(kp2[:], ksw[:], s2t[:])
                        nc.gpsimd.tensor_add(kpe[t][:], kp1[:], kp2[:])
                        pend.append((t, qcn_t, ckvn_t))
                        if len(pend) > 1:
                            do_tp1(*pend.pop(0))
                    for args in pend:
                        do_tp1(*args)

                # ---------------- stage 2: qb / kvb + rope + assemble -----
                acts2 = top.enter_context(tc.tile_pool(name="acts2", bufs=1))
                qfT = [acts2.tile([128, S], f32r, tag=f"qfT{j}", name=f"qfT{j}")
                       for j in range(2)]
                kfT = [acts2.tile([128, S], f32r, tag=f"kfT{j}", name=f"kfT{j}")
                       for j in range(2)]
                kvd = [acts2.tile([128, KVW], f32r, tag=f"kvd{t}", name=f"kvd{t}")
                       for t in range(TT)]
                wqb_t = wq2.tile([128, RT, DPC], f32r, tag="wqb")
                nc.sync.dma_start(
                    wqb_t[:], wqb_d.rearrange("(k p) n -> p k n", p=128))
                wkvb_t = wq2.tile([128, RT, KVW], f32r, tag="wkvb")
                nc.sync.dma_start(
                    wkvb_t[:], wkvb_d.rearrange("(k p) n -> p k n", p=128))
                wout_t = wq2.tile([128, 2, E], f32r, tag="wout")
                nc.sync.dma_start(
                    wout_t[:], wout_d.rearrange("(k p) n -> p k n", p=128))

                with contextlib.ExitStack() as st2:
                    pq = st2.enter_context(
                        tc.tile_pool(name="pq", bufs=2, space="PSUM"))
                    pkv = st2.enter_context(
                        tc.tile_pool(name="pkv", bufs=2, space="PSUM"))
                    ptp2 = st2.enter_context(
                        tc.tile_pool(name="ptp2", bufs=4, space="PSUM"))
                    ab = st2.enter_context(tc.tile_pool(name="ab", bufs=3))
                    qk2 = st2.enter_context(tc.tile_pool(name="qk2", bufs=2))

                    pend2 = []

                    def do_tp2(t, qf_t, kf_t):
                        ts_ = slice(t * 128, (t + 1) * 128)
                        for j in range(2):
                            js = slice(j * 128, (j + 1) * 128)
                            tp = ptp2.tile([128, 128], f32r, tag="tp2",
                                           name=f"tpq2_{t}_{j}")
                            nc.tensor.transpose(tp[:], qf_t[:, js], ident_t[:])
                            nc.scalar.activation(qfT[j][:, ts_], tp[:], AF.Copy)
                            tp = ptp2.tile([128, 128], f32r, tag="tp2",
                                           name=f"tpk2_{t}_{j}")
                            nc.tensor.transpose(tp[:], kf_t[:, js], ident_t[:])
                            nc.vector.tensor_copy(kfT[j][:, ts_], tp[:])

                    for t in range(TT):
                        ts_ = slice(t * 128, (t + 1) * 128)
                        # qb
                        p_q = pq.tile([128, DPC], f32, tag="p_q")
                        for r in range(RT):
                            nc.tensor.matmul(p_q[:], qcnT[r][:, ts_],
                                             wqb_t[:, r, :],
                                             start=(r == 0), stop=False)
                        nc.tensor.matmul(p_q[:], ones1_t[:], qconst_t[:],
                                         start=False, stop=True)
                        q_t = qk2.tile([128, DPC], f32r, tag="q_t")
                        nc.scalar.activation(q_t[:], p_q[:], AF.Copy)
                        # rope on q
                        a_t = ab.tile([128, DPC], f32r, tag="a_t")
                        nc.sync.dma_start(a_t[:], aq_d[ts_, :])
                        b_t = ab.tile([128, DPC], f32r, tag="b_t")
                        nc.sync.dma_start(b_t[:], bq_d[ts_, :])
                        q_sw = qk2.tile([128, DPC], f32r, tag="q_sw")
                        q3 = q_t.rearrange("p (i two) -> p i two", two=2)
                        qs3 = q_sw.rearrange("p (i two) -> p i two", two=2)
                        nc.gpsimd.tensor_copy(qs3[:, :, 0:1], q3[:, :, 1:2])
                        nc.gpsimd.tensor_copy(qs3[:, :, 1:2], q3[:, :, 0:1])
                        qt1 = qk2.tile([128, DPC], f32r, tag="qt1")
                        nc.vector.tensor_mul(qt1[:], q_t[:], a_t[:])
                        qt2 = qk2.tile([128, DPC], f32r, tag="qt2")
                        nc.vector.tensor_mul(qt2[:], q_sw[:], b_t[:])
                        qf_t = qk2.tile([128, DPC], f32r, tag="qf_t", bufs=3)
                        nc.vector.tensor_add(qf_t[:], qt1[:], qt2[:])
                        qf_pend = (t, qf_t)
                        # kvb
                        p_kv = pkv.tile([128, KVW], f32, tag="p_kv")
                        for r in range(RT):
                            nc.tensor.matmul(p_kv[:], ckvnT[r][:, ts_],
                                             wkvb_t[:, r, :],
                                             start=(r == 0), stop=False)
                        nc.tensor.matmul(p_kv[:], ones1_t[:], kconst_t[:],
                                         start=False, stop=True)
                        nc.scalar.activation(kvd[t][:], p_kv[:], AF.Copy)
                        kvd3 = kvd[t].rearrange("p (h c) -> p h c", h=HPC)
                        nc.gpsimd.tensor_copy(
                            kvd3[:, :, 96:97],
                            ones4_t.rearrange("p (h o) -> p h o", o=1)[:])
                        # k_full assembly
                        kf_t = qk2.tile([128, DPC], f32r, tag="kf_t", bufs=3)
                        kf3 = kf_t.rearrange("p (h c) -> p h c", h=HPC)
                        nc.gpsimd.tensor_copy(kf3[:, :, 0:32],
                                               kvd3[:, :, 0:32])
                        for h in range(HPC):
                            nc.gpsimd.tensor_copy(
                                kf_t[:, h * 64 + 32:h * 64 + 64], kpe[t][:])
                        pend2.append((qf_pend[0], qf_pend[1], kf_t))
                        if len(pend2) > 1:
                            do_tp2(*pend2.pop(0))
                    for args in pend2:
                        do_tp2(*args)

                # ---------------- stage 3+4: attention + out projection ---
                with contextlib.ExitStack() as st3:
                    ps_s = st3.enter_context(
                        tc.tile_pool(name="ps_s", bufs=4, space="PSUM"))
                    ps_av = st3.enter_context(
                        tc.tile_pool(name="ps_av", bufs=2, space="PSUM"))
                    ps_o = st3.enter_context(
                        tc.tile_pool(name="ps_o", bufs=2, space="PSUM"))
                    ex = st3.enter_context(tc.tile_pool(name="ex", bufs=6))
                    on = st3.enter_context(tc.tile_pool(name="on", bufs=4))
                    ozs = st3.enter_context(tc.tile_pool(name="ozs", bufs=4))
                    osb = st3.enter_context(tc.tile_pool(name="osb", bufs=2))

                    pend3 = []

                    def do_outproj(sb_i, onorm):
                        for tc_i in range(4):
                            tcs = slice(tc_i * 128, (tc_i + 1) * 128)
                            o_t = osb.tile([128, E], f32, tag="o_t",
                                           name=f"o_t_{sb_i}_{tc_i}")
                            for ei in range(2):
                                es = slice(ei * 512, (ei + 1) * 512)
                                p_o = ps_o.tile([128, 512], f32, tag="p_o",
                                                name=f"p_o_{sb_i}_{tc_i}_{ei}")
                                for kk in range(2):
                                    nc.tensor.matmul(
                                        p_o[:], onorm[kk][:, tcs],
                                        wout_t[:, kk, es],
                                        start=(kk == 0), stop=(kk == 1))
                                nc.vector.tensor_copy(o_t[:, es], p_o[:])
                            nc.sync.dma_start(
                                out_d[sb_i * 512 + tc_i * 128:
                                      sb_i * 512 + tc_i * 128 + 128, :],
                                o_t[:])

                    for sb_i in range(SB):
                        ss = slice(sb_i * 512, (sb_i + 1) * 512)
                        onorm = [on.tile([128, 512], f32r, tag=f"on{j}",
                                         name=f"on{j}_{sb_i}")
                                 for j in range(2)]
                        for h in range(HPC):
                            j, half = h // 2, (h % 2) * 64
                            hs = slice(half, half + 64)
                            p_av = ps_av.tile([128, 512], f32, tag="p_av")
                            for u in range(UT):
                                us = slice(u * 128, (u + 1) * 128)
                                p_s = ps_s.tile([128, 512], f32, tag="p_s")
                                nc.tensor.matmul(p_s[:], kfT[j][hs, us],
                                                 qfT[j][hs, ss],
                                                 start=True, stop=True)
                                e_t = ex.tile([128, 512], f32r, tag="e_t")
                                nc.scalar.activation(e_t[:], p_s[:], AF.Exp,
                                                     scale=0.125)
                                nc.tensor.matmul(
                                    p_av[0:65, :],
                                    kvd[u][:, h * 97 + 32:h * 97 + 97],
                                    e_t[:], start=(u == 0), stop=(u == UT - 1))
                            rz = ozs.tile([1, 512], f32r, tag="rz")
                            with nc.allow_low_precision(reason="f32r Z"):
                                nc.vector.reciprocal(rz[:], p_av[64:65, :])
                            zb = ozs.tile([64, 512], f32r, tag="zb")
                            nc.gpsimd.partition_broadcast(zb[:], rz[:],
                                                          channels=64)
                            nc.vector.tensor_mul(onorm[h // 2][hs, :],
                                                 p_av[0:64, :], zb[:])
                        pend3.append((sb_i, onorm))
                        if len(pend3) > 1:
                            do_outproj(*pend3.pop(0))
                    for args in pend3:
                        do_outproj(*args)

            if reps == 1:
                body()
            else:
                with tc.For_i(0, reps, 1):
                    body()

    nc.compile()
    return nc


def _host_prep(x, Wqa, g_qa, b_qa, Wqb, Wkva, g_kva, b_kva, Wkvb, Wout):
    f32 = np.float32
    x = np.asarray(x, f32)
    Wqa = np.asarray(Wqa, f32); Wqb = np.asarray(Wqb, f32)
    Wkva = np.asarray(Wkva, f32); Wkvb = np.asarray(Wkvb, f32)
    Wout = np.asarray(Wout, f32)
    g_qa = np.asarray(g_qa, f32); b_qa = np.asarray(b_qa, f32)
    g_kva = np.asarray(g_kva, f32); b_kva = np.asarray(b_kva, f32)

    inv = 1.0 / (10000.0 ** (np.arange(0, ROPE, 2, dtype=f32) / ROPE))
    fr = np.arange(S, dtype=f32)[:, None] * inv[None, :]
    cos, sin = np.cos(fr).astype(f32), np.sin(fr).astype(f32)
    c2 = np.repeat(cos, 2, axis=1)
    s2 = np.empty((S, ROPE), f32)
    s2[:, 0::2] = -sin
    s2[:, 1::2] = sin
    Aq = np.ones((S, DPC), f32)
    Bq = np.zeros((S, DPC), f32)
    for h in range(HPC):
        Aq[:, h * 64 + 32:h * 64 + 64] = c2
        Bq[:, h * 64 + 32:h * 64 + 64] = s2

    shared = {
        "WqaT": np.ascontiguousarray(Wqa.T),
        "WkvaT": np.ascontiguousarray(Wkva.T),
        "Aq": Aq, "Bq": Bq, "c2k": c2, "s2k": s2,
        "ident": np.eye(128, dtype=f32),
        "ones1": np.ones((1, 128), f32),
        "ones4": np.ones((128, 4), f32),
        "epst": np.full((128, 1), EPS, f32),
    }
    in_maps = []
    for core in range(NCORES):
        b, hg = core // HPC, core % HPC
        Wqb_sl = Wqb[hg * DPC:(hg + 1) * DPC, :]
        WkvbT_eff = np.zeros((KVL, KVW), f32)
        kconst = np.zeros((1, KVW), f32)
        for h in range(HPC):
            blk = Wkvb[(hg * HPC + h) * 96:(hg * HPC + h + 1) * 96, :] \
                * g_kva[None, :]
            WkvbT_eff[:, h * 97:h * 97 + 96] = blk.T
            kconst[0, h * 97:h * 97 + 96] = b_kva @ blk.T
        m = dict(shared)
        m["xT"] = np.ascontiguousarray(x[b].T)
        m["WqbT"] = np.ascontiguousarray((Wqb_sl * g_qa[None, :]).T)
        m["qconst"] = (b_qa @ Wqb_sl.T)[None, :].astype(f32)
        m["WkvbT"] = WkvbT_eff
        m["kconst"] = kconst
        m["WoutT"] = np.ascontiguousarray(
            Wout[:, hg * DPC:(hg + 1) * DPC].T)
        in_maps.append(m)
    return in_maps


def kernel(**inputs):
    from concourse.bass_utils import run_bass_kernel_spmd
    if "nc" not in _CACHE:
        _CACHE["nc"] = _build(reps=1)
    nc = _CACHE["nc"]
    in_maps = _host_prep(**inputs)
    res = run_bass_kernel_spmd(nc, in_maps, core_ids=list(range(NCORES)))
    out = np.zeros((B, S, E), np.float32)
    for core in range(NCORES):
        out[core // HPC] += res.results[core]["out"]
    return out

